# revision 43
# baseline (speedup 1.0000x reference)
"""Trainium2 Bass kernel for nn_ClearMeshLoss.

Sharding: pred-point axis (N=8192) split 8 ways; each core computes
  - its 1024x8192 slab of the (negated) pairwise sq-dist matrix via PE
    matmuls (K=5 lift, host negates the gt lift so PSUM = -dist),
  - slab staged to SBUF in bf16 (ACT cast) -> DVE 2x throughput,
  - row max (= -row min) via bf16 halving folds to per-tile maxima; the
    winning 512-wide tile is found with an is_equal/iota-min search,
    re-fetched from a DRAM spill by one per-partition indirect DMA, and
    searched for the exact argmax column (software-pipelined across
    i-blocks, two iterations deep),
  - column-max partials accumulated in bf16, reduced across partitions
    via PE transposes + DVE reduces, combined across cores on host,
  - normal-consistency cosines via one indirect-DMA gather of matched
    gt normals,
  - its slice of the SDF L1 sum,
  - edge-sharpness / watertight terms on the gpsimd engine (overlapped
    with the DVE chamfer work): host supplies only a lexsort ORDERING of
    the 120k edge keys (plus gathered per-edge face-vertex layout); the
    device verifies sortedness and computes face normals, dihedral
    cosines, run-length counts, and all sums. A sort-order violation
    raises at runtime.
"""
import numpy as np

import concourse.bass as bass
import concourse.mybir as mybir
import concourse.tile as tile
from concourse import bacc
from concourse.bass_utils import run_bass_kernel_spmd

P = 128
N = 8192          # pred points (total)
M = 8192          # gt points
NC_CORES = 8
NPC = N // NC_CORES          # 1024 pred rows per core
IB = NPC // P                # 8 i-blocks per core
NS = 65536
NSC = NS // NC_CORES         # 8192 sdf elems per core
V = 20000
F = 40000

CHAMFER_W, NORMAL_W, EDGE_W, WATERTIGHT_W, SDF_W = 1.0, 0.5, 0.3, 0.2, 1.0
DIHEDRAL_THRESHOLD = 0.5
EPS_COS = 1e-8
EPS_NRM = 1e-12

# edge pipeline: 3F = 120000 edges padded to 2^17, laid out [128, 1024] with a
# 3-column overlap so run/pair/cos windows never cross partitions
TE = 3 * F                 # 120000 real edges
TEP = 131072               # padded
EW = TEP // P              # 1024 own columns per partition
EWo = EW + 3               # own + 3 overlap columns (host-side full layout)
EWC = EW // NC_CORES       # 128 own columns per partition per core
EWoC = EWC + 3             # per-core slice width

KERNEL_TRACE = False
TRACE_SINK = None
_CACHED_NC = None

f32 = mybir.dt.float32
f32r = mybir.dt.float32r
bf16 = mybir.dt.bfloat16
i32 = mybir.dt.int32
Alu = mybir.AluOpType
Ax = mybir.AxisListType
Act = mybir.ActivationFunctionType


def _build_program():
    from concourse.masks import make_identity

    nc = bacc.Bacc("TRN2", target_bir_lowering=False, debug=False,
                   num_devices=NC_CORES)

    # ---- I/O ----
    p5 = nc.dram_tensor("p5", [5, NPC], f32r, kind="ExternalInput")
    g5 = nc.dram_tensor("g5", [5, M], f32r, kind="ExternalInput")  # negated
    pn = nc.dram_tensor("pn", [NPC, 3], f32, kind="ExternalInput")
    gnrm = nc.dram_tensor("gnrm", [M, 4], f32, kind="ExternalInput")
    ps = nc.dram_tensor("ps", [P, NSC // P], f32, kind="ExternalInput")
    gs = nc.dram_tensor("gs", [P, NSC // P], f32, kind="ExternalInput")

    elo = nc.dram_tensor("elo", [P, EWoC], i32, kind="ExternalInput")
    ehi = nc.dram_tensor("ehi", [P, EWoC], i32, kind="ExternalInput")
    eid = nc.dram_tensor("eid", [P, EWoC], i32, kind="ExternalInput")
    vfs = nc.dram_tensor("vfs", [P, EWoC, 9], f32, kind="ExternalInput")

    rowmin_o = nc.dram_tensor("rowmin", [P, IB], f32, kind="ExternalOutput")
    epart_o = nc.dram_tensor("epart", [P, 4], f32, kind="ExternalOutput")
    sabs_o = nc.dram_tensor("sabs", [P, 1], f32, kind="ExternalOutput")
    colmax_o = nc.dram_tensor("colmax", [P, M // P], f32, kind="ExternalOutput")
    sdfsum_o = nc.dram_tensor("sdfsum", [P, 1], f32, kind="ExternalOutput")

    # DRAM scratch: negated dist slab, bf16, row (ib*P + p) = 8192 cols
    dist_dram = nc.dram_tensor("dist_scratch", [IB * P, M], bf16,
                               kind="Internal")

    with tile.TileContext(nc) as tc:
        with (
            tc.tile_pool(name="const", bufs=1) as cpool,
            tc.tile_pool(name="swork", bufs=2) as swork,
            tc.tile_pool(name="ssm", bufs=2) as ssm,
            tc.tile_pool(name="ep", bufs=1) as ep,
        ):
            # ---- chamfer inputs first: matmuls are the critical path ----
            p5_sb = cpool.tile([5, NPC], f32r)
            nc.sync.dma_start(p5_sb[:], p5.ap())
            g5_sb = cpool.tile([5, M], f32r)
            nc.sync.dma_start(g5_sb[:], g5.ap())
            pn_sb = ssm.tile([P, IB, 3], f32)
            nc.sync.dma_start(pn_sb[:], pn.ap().rearrange("(p q) d -> p q d", p=P))
            ps_sb = ssm.tile([P, NSC // P], f32)
            gs_sb = ssm.tile([P, NSC // P], f32)
            nc.sync.dma_start(ps_sb[:], ps.ap())
            nc.sync.dma_start(gs_sb[:], gs.ap())

            elo_t = ep.tile([P, EWoC], i32)
            ehi_t = ep.tile([P, EWoC], i32)
            eid_t = ep.tile([P, EWoC], i32)
            vfs_t = ep.tile([P, EWoC, 9], f32)
            nc.sync.dma_start(elo_t[:], elo.ap())
            nc.sync.dma_start(ehi_t[:], ehi.ap())
            nc.sync.dma_start(eid_t[:], eid.ap())
            nc.sync.dma_start(vfs_t[:], vfs.ap())

            # ---- constants (gpsimd; cheap, early) ----
            it512_i = cpool.tile([P, 512], i32)
            nc.gpsimd.iota(it512_i[:], [[1, 512]], channel_multiplier=0)
            iota512MB = cpool.tile([P, 512], f32)   # iota - 1024
            nc.gpsimd.tensor_copy(iota512MB[:], it512_i[:])
            nc.gpsimd.tensor_scalar(out=iota512MB[:], in0=iota512MB[:],
                                    scalar1=1024.0, scalar2=None,
                                    op0=Alu.subtract)
            it16_i = cpool.tile([P, 16], i32)
            nc.gpsimd.iota(it16_i[:], [[1, 16]], channel_multiplier=0)
            iota16MB = cpool.tile([P, 16], f32)     # iota - 1024
            nc.gpsimd.tensor_copy(iota16MB[:], it16_i[:])
            nc.gpsimd.tensor_scalar(out=iota16MB[:], in0=iota16MB[:],
                                    scalar1=1024.0, scalar2=None,
                                    op0=Alu.subtract)
            rowb_i = cpool.tile([P, 1], i32)        # p*16 win-row base
            nc.gpsimd.iota(rowb_i[:], [[1, 1]], channel_multiplier=16)
            rowb_f = cpool.tile([P, 1], f32)
            nc.gpsimd.tensor_copy(rowb_f[:], rowb_i[:])
            eps4 = cpool.tile([P, 1], f32)
            nc.gpsimd.memset(eps4[:], 1e-24)
            identf = cpool.tile([P, P], f32)
            make_identity(nc, identf[:])
            identb = cpool.tile([P, P], bf16)
            nc.gpsimd.tensor_copy(identb[:], identf[:])

            # ---- edge phase part A: int/compare prefix on DVE (fills the
            # ---- DVE while the first matmuls run), geometry chain on Pool
            W1 = EWoC - 1  # 130
            dlo = ep.tile([P, W1], i32, tag="ti1")
            nc.vector.tensor_tensor(out=dlo[:], in0=elo_t[:, 1:],
                                    in1=elo_t[:, :-1], op=Alu.not_equal)
            dhi = ep.tile([P, W1], i32, tag="ti2")
            nc.vector.tensor_tensor(out=dhi[:], in0=ehi_t[:, 1:],
                                    in1=ehi_t[:, :-1], op=Alu.not_equal)
            rs = ep.tile([P, W1], i32, tag="rs")
            nc.vector.tensor_tensor(out=rs[:], in0=dlo[:], in1=dhi[:],
                                    op=Alu.logical_or)
            notr = ep.tile([P, W1], i32, tag="ti2")
            nc.vector.tensor_scalar(out=notr[:], in0=rs[:], scalar1=-1,
                                    scalar2=1, op0=Alu.mult, op1=Alu.add)
            p2 = ep.tile([P, EWC], i32, tag="p2")
            nc.vector.tensor_tensor(out=p2[:], in0=rs[:, 0:EWC],
                                    in1=notr[:, 1:EWC + 1], op=Alu.logical_and)
            nc.vector.tensor_tensor(out=p2[:], in0=p2[:], in1=rs[:, 2:EWC + 2],
                                    op=Alu.logical_and)
            totali = ep.tile([P, 1], i32, tag="s1")
            with nc.allow_low_precision(reason="exact small-int counts"):
                nc.vector.tensor_reduce(out=totali[:], in_=rs[:, 0:EWC],
                                        axis=Ax.X, op=Alu.add)
            p2f = ep.tile([P, EWC], f32, tag="p2f")
            nc.vector.tensor_copy(p2f[:], p2[:])

            lt1 = ep.tile([P, EWC], i32, tag="ti1")
            nc.vector.tensor_tensor(out=lt1[:], in0=elo_t[:, 1:EWC + 1],
                                    in1=elo_t[:, 0:EWC], op=Alu.is_lt)
            eq1 = ep.tile([P, EWC], i32, tag="ti3")
            nc.vector.tensor_tensor(out=eq1[:], in0=elo_t[:, 1:EWC + 1],
                                    in1=elo_t[:, 0:EWC], op=Alu.is_equal)
            lt2 = ep.tile([P, EWC], i32, tag="ti2")
            nc.vector.tensor_tensor(out=lt2[:], in0=ehi_t[:, 1:EWC + 1],
                                    in1=ehi_t[:, 0:EWC], op=Alu.is_lt)
            nc.vector.tensor_tensor(out=eq1[:], in0=eq1[:], in1=lt2[:],
                                    op=Alu.logical_and)
            nc.vector.tensor_tensor(out=eq1[:], in0=eq1[:], in1=lt1[:],
                                    op=Alu.logical_or)
            violi = ep.tile([P, 1], i32, tag="s2")
            with nc.allow_low_precision(reason="exact small-int counts"):
                nc.vector.tensor_reduce(out=violi[:], in_=eq1[:], axis=Ax.X,
                                        op=Alu.add)

            eidf = ep.tile([P, EWoC], f32, tag="tf1")
            nc.vector.tensor_copy(eidf[:], eid_t[:])
            nc.vector.tensor_scalar(out=eidf[:], in0=eidf[:], scalar1=-1.0,
                                    scalar2=0.33333334, op0=Alu.add,
                                    op1=Alu.mult)
            fidi = ep.tile([P, EWoC], i32, tag="ti4")
            nc.vector.tensor_copy(fidi[:], eidf[:])
            samef = ep.tile([P, EWC], i32, tag="ti1")
            nc.vector.tensor_tensor(out=samef[:], in0=fidi[:, 1:EWC + 1],
                                    in1=fidi[:, 2:EWC + 2], op=Alu.is_equal)
            samef_f = ep.tile([P, EWC], f32, tag="tf2")
            nc.vector.tensor_copy(samef_f[:], samef[:])
            # XLA-FMA artifact emulation: degenerate face with v1==v2 gets a
            # unit normal in the reference, so a self-paired edge scores 0.5
            eqv = ep.tile([P, EWoC, 3], f32, tag="eq3")
            nc.vector.tensor_tensor(out=eqv[:], in0=vfs_t[:, :, 3:6],
                                    in1=vfs_t[:, :, 6:9], op=Alu.is_equal)
            alleq = ep.tile([P, EWoC], f32, tag="tf3")
            nc.vector.tensor_reduce(out=alleq[:], in_=eqv[:], axis=Ax.X,
                                    op=Alu.min)

            # Pool geometry chain (input/DVE-prefix dependent only)
            ovr = ep.tile([P, EWC], f32, tag="tf4")
            nc.gpsimd.tensor_tensor(out=ovr[:], in0=samef_f[:],
                                    in1=alleq[:, 1:EWC + 1], op=Alu.mult)
            e1t = ep.tile([P, EWoC, 3], f32, tag="e1")
            nc.gpsimd.tensor_tensor(out=e1t[:], in0=vfs_t[:, :, 3:6],
                                    in1=vfs_t[:, :, 0:3], op=Alu.subtract)
            e2t = ep.tile([P, EWoC, 3], f32, tag="e2")
            nc.gpsimd.tensor_tensor(out=e2t[:], in0=vfs_t[:, :, 6:9],
                                    in1=vfs_t[:, :, 0:3], op=Alu.subtract)
            n3 = ep.tile([P, EWoC, 3], f32, tag="n3")
            for k in range(3):
                ka, kb = (k + 1) % 3, (k + 2) % 3
                m1 = ep.tile([P, EWoC], f32, tag="tm1")
                m2 = ep.tile([P, EWoC], f32, tag="tm2")
                nc.gpsimd.tensor_tensor(out=m1[:], in0=e1t[:, :, ka],
                                        in1=e2t[:, :, kb], op=Alu.mult)
                nc.gpsimd.tensor_tensor(out=m2[:], in0=e1t[:, :, kb],
                                        in1=e2t[:, :, ka], op=Alu.mult)
                nc.gpsimd.tensor_tensor(out=n3[:, :, k], in0=m1[:], in1=m2[:],
                                        op=Alu.subtract)
            nsq = ep.tile([P, EWoC], f32, tag="tm3")
            nc.gpsimd.tensor_tensor(out=nsq[:], in0=n3[:, :, 0],
                                    in1=n3[:, :, 0], op=Alu.mult)
            for k in (1, 2):
                mk = ep.tile([P, EWoC], f32, tag="tm1")
                nc.gpsimd.tensor_tensor(out=mk[:], in0=n3[:, :, k],
                                        in1=n3[:, :, k], op=Alu.mult)
                nc.gpsimd.tensor_tensor(out=nsq[:], in0=nsq[:], in1=mk[:],
                                        op=Alu.add)
            # |n1|^2 |n2|^2 product of adjacent entries, then unnormalized dot
            nsqp = ep.tile([P, EWC], f32, tag="tf5")
            nc.gpsimd.tensor_tensor(out=nsqp[:], in0=nsq[:, 1:EWC + 1],
                                    in1=nsq[:, 2:EWC + 2], op=Alu.mult)
            dotu = ep.tile([P, EWC], f32, tag="tf6")
            du1 = ep.tile([P, EWC], f32, tag="tm1")
            nc.gpsimd.tensor_tensor(out=dotu[:], in0=n3[:, 1:EWC + 1, 0],
                                    in1=n3[:, 2:EWC + 2, 0], op=Alu.mult)
            for k in (1, 2):
                nc.gpsimd.tensor_tensor(out=du1[:], in0=n3[:, 1:EWC + 1, k],
                                        in1=n3[:, 2:EWC + 2, k], op=Alu.mult)
                nc.gpsimd.tensor_tensor(out=dotu[:], in0=dotu[:], in1=du1[:],
                                        op=Alu.add)

            # ---- sdf L1 partial (DVE, input-dependent only) ----
            sdiff = ssm.tile([P, NSC // P], f32)
            nc.vector.tensor_tensor(out=sdiff[:], in0=ps_sb[:], in1=gs_sb[:],
                                    op=Alu.subtract)
            sdfsum = ssm.tile([P, 1], f32)
            nc.vector.tensor_reduce(out=sdfsum[:], in_=sdiff[:], axis=Ax.X,
                                    op=Alu.add, apply_absolute_value=True)
            nc.sync.dma_start(sdfsum_o.ap(), sdfsum[:])

            # ---- chamfer loop with software-pipelined argmax tail ----
            colacc = cpool.tile([P, M], bf16)
            rowmax8 = cpool.tile([P, IB], f32)
            tstar8 = cpool.tile([P, IB], f32)
            wins = []
            for i in range(IB):
                win_t = cpool.tile([P, 512], bf16, tag=f"win{i}", name=f"win{i}")
                wins.append(win_t)
            matched4 = ssm.tile([P, IB, 4], f32)
            matched = matched4[:, :, 0:3]
            dotn = ssm.tile([P, IB], f32)
            pnn = ssm.tile([P, IB], f32)
            gnn = ssm.tile([P, IB], f32)
            cosv = ssm.tile([P, IB], f32)
            tmp3 = ssm.tile([P, IB, 3], f32)

            def emit_normal(lo, hi):
                s = slice(lo, hi)
                nc.vector.tensor_tensor(out=tmp3[:, s, :], in0=pn_sb[:, s, :],
                                        in1=matched[:, s, :], op=Alu.mult)
                nc.vector.tensor_reduce(out=dotn[:, s], in_=tmp3[:, s, :],
                                        axis=Ax.X, op=Alu.add)
                nc.vector.tensor_tensor(out=tmp3[:, s, :], in0=pn_sb[:, s, :],
                                        in1=pn_sb[:, s, :], op=Alu.mult)
                nc.vector.tensor_reduce(out=pnn[:, s], in_=tmp3[:, s, :],
                                        axis=Ax.X, op=Alu.add)
                nc.scalar.activation(pnn[:, s], pnn[:, s], Act.Sqrt)
                nc.vector.tensor_scalar(out=pnn[:, s], in0=pnn[:, s],
                                        scalar1=EPS_COS, scalar2=None,
                                        op0=Alu.max)
                nc.vector.tensor_tensor(out=tmp3[:, s, :], in0=matched[:, s, :],
                                        in1=matched[:, s, :], op=Alu.mult)
                nc.vector.tensor_reduce(out=gnn[:, s], in_=tmp3[:, s, :],
                                        axis=Ax.X, op=Alu.add)
                nc.scalar.activation(gnn[:, s], gnn[:, s], Act.Sqrt)
                nc.vector.tensor_scalar(out=gnn[:, s], in0=gnn[:, s],
                                        scalar1=EPS_COS, scalar2=None,
                                        op0=Alu.max)
                nc.vector.tensor_tensor(out=gnn[:, s], in0=pnn[:, s],
                                        in1=gnn[:, s], op=Alu.mult)
                nc.vector.reciprocal(gnn[:, s], gnn[:, s])
                nc.vector.tensor_tensor(out=cosv[:, s], in0=dotn[:, s],
                                        in1=gnn[:, s], op=Alu.mult)

            def emit_wg(jb):
                # win-tile row = p*16 + t* + 1024 + jb*2048; gather the tile
                ridx_f = swork.tile([P, 1], f32, tag="ridx_f", name="ridx_f")
                nc.vector.scalar_tensor_tensor(
                    out=ridx_f[:], in0=tstar8[:, jb:jb + 1],
                    scalar=float(1024 + jb * 2048), in1=rowb_f[:],
                    op0=Alu.add, op1=Alu.add)
                ridx_i = swork.tile([P, 1], i32, tag="ridx_i", name="ridx_i")
                nc.vector.tensor_copy(ridx_i[:], ridx_f[:])
                nc.gpsimd.indirect_dma_start(
                    out=wins[jb][:], out_offset=None,
                    in_=dist_dram.ap().rearrange("r (t c) -> (r t) c", c=512),
                    in_offset=bass.IndirectOffsetOnAxis(ap=ridx_i[:, :1],
                                                        axis=0))

            def emit_tail(jb):
                # w* within winning tile; nnidx; gather matched gt normal
                cw = swork.tile([P, 512], f32, tag="cw", name="cw")
                nc.vector.scalar_tensor_tensor(
                    out=cw[:], in0=wins[jb][:],
                    scalar=rowmax8[:, jb:jb + 1], in1=iota512MB[:],
                    op0=Alu.is_equal, op1=Alu.mult)
                ws = swork.tile([P, 1], f32, tag="ws", name="ws")
                nc.vector.tensor_reduce(out=ws[:], in_=cw[:], axis=Ax.X,
                                        op=Alu.min)
                # nnidx = t*_raw*512 + w*_raw + (1024*512 + 1024)
                nnf = swork.tile([P, 1], f32, tag="nnf", name="nnf")
                nc.vector.scalar_tensor_tensor(
                    out=nnf[:], in0=tstar8[:, jb:jb + 1], scalar=512.0,
                    in1=ws[:], op0=Alu.mult, op1=Alu.add)
                nc.vector.tensor_scalar(out=nnf[:], in0=nnf[:],
                                        scalar1=float(1024 * 512 + 1024),
                                        scalar2=None, op0=Alu.add)
                nni = swork.tile([P, 1], i32, tag="nni", name="nni")
                nc.vector.tensor_copy(nni[:], nnf[:])
                mg = swork.tile([P, 4], f32, tag=f"mg{jb}", name="mg")
                nc.gpsimd.indirect_dma_start(
                    out=mg[:], out_offset=None, in_=gnrm.ap(),
                    in_offset=bass.IndirectOffsetOnAxis(ap=nni[:, :1], axis=0))
                nc.vector.tensor_copy(matched4[:, jb:jb + 1, :], mg[:, None, :])

            def emit_edge_b():
                # sqrt(|n1|^2 |n2|^2 + eps^4): eps bias keeps 1/x finite for
                # exactly-degenerate faces (reference clamps |n| to 1e-12)
                snp = ep.tile([P, EWC], f32, tag="tf7", name="snp")
                nc.scalar.activation(snp[:], nsqp[:], Act.Sqrt,
                                     bias=eps4[:, :1])
                rec = ep.tile([P, EWC], f32, tag="tf8", name="rec")
                nc.vector.reciprocal(rec[:], snp[:])
                cose = ep.tile([P, EWC], f32, tag="tf7")
                nc.gpsimd.tensor_tensor(out=cose[:], in0=dotu[:], in1=rec[:],
                                        op=Alu.mult)
                return cose

            def emit_edge_c(cose):
                cosa = ep.tile([P, EWC], f32, tag="tf1")
                nc.vector.tensor_scalar(out=cosa[:], in0=cose[:], scalar1=-0.5,
                                        scalar2=0.0, op0=Alu.add, op1=Alu.max)
                d5 = ep.tile([P, EWC], f32, tag="tf3")
                nc.vector.tensor_scalar(out=d5[:], in0=cosa[:], scalar1=-1.0,
                                        scalar2=0.5, op0=Alu.mult, op1=Alu.add)
                nc.vector.tensor_tensor(out=d5[:], in0=d5[:], in1=ovr[:],
                                        op=Alu.mult)
                nc.vector.tensor_tensor(out=cosa[:], in0=cosa[:], in1=d5[:],
                                        op=Alu.add)
                nc.vector.tensor_tensor(out=cosa[:], in0=cosa[:], in1=p2f[:],
                                        op=Alu.mult)
                spart = ep.tile([P, 1], f32, tag="s3")
                nc.vector.tensor_reduce(out=spart[:], in_=cosa[:], axis=Ax.X,
                                        op=Alu.add)
                cnt2p = ep.tile([P, 1], f32, tag="s4")
                nc.vector.tensor_reduce(out=cnt2p[:], in_=p2f[:], axis=Ax.X,
                                        op=Alu.add)
                epk = ep.tile([P, 4], f32, tag="s5")
                nc.vector.tensor_copy(epk[:, 0:1], totali[:])
                nc.vector.tensor_copy(epk[:, 1:2], cnt2p[:])
                nc.vector.tensor_copy(epk[:, 2:3], spart[:])
                nc.vector.tensor_copy(epk[:, 3:4], violi[:])
                nc.sync.dma_start(epart_o.ap(), epk[:])

            cose = None
            with (
                tc.tile_pool(name="sbig", bufs=4) as sbig,
                tc.tile_pool(name="psum", bufs=2, space="PSUM") as pp,
            ):
                for ib in range(IB):
                    slab = sbig.tile([P, M], bf16, tag="slab")
                    h1 = swork.tile([P, 16, 256], bf16, tag="h1", name="h1")
                    for q in range(4):
                        d_ps = pp.tile([P, 2048], f32, tag="dps")
                        for s in range(4):
                            j0 = (q * 4 + s) * 512
                            nc.tensor.matmul(d_ps[:, s * 512:(s + 1) * 512],
                                             lhsT=p5_sb[:, ib * P:(ib + 1) * P],
                                             rhs=g5_sb[:, j0:j0 + 512],
                                             start=True, stop=True)
                        nc.scalar.activation(slab[:, q * 2048:(q + 1) * 2048],
                                             d_ps[:], Act.Copy)
                        if ib == 0:
                            nc.vector.tensor_copy(
                                colacc[:, q * 2048:(q + 1) * 2048],
                                slab[:, q * 2048:(q + 1) * 2048])
                        else:
                            nc.vector.tensor_tensor(
                                out=colacc[:, q * 2048:(q + 1) * 2048],
                                in0=colacc[:, q * 2048:(q + 1) * 2048],
                                in1=slab[:, q * 2048:(q + 1) * 2048],
                                op=Alu.max)
                    # per-tile maxima via halving folds (bf16 2x)
                    s3 = slab[:].rearrange("p (t c) -> p t c", t=16)
                    nc.vector.tensor_tensor(out=h1[:], in0=s3[:, :, 0:256],
                                            in1=s3[:, :, 256:512], op=Alu.max)
                    h2 = swork.tile([P, 16, 128], bf16, tag="h2")
                    nc.vector.tensor_tensor(out=h2[:], in0=h1[:, :, 0:128],
                                            in1=h1[:, :, 128:256], op=Alu.max)
                    h3 = swork.tile([P, 16, 64], bf16, tag="h3")
                    nc.vector.tensor_tensor(out=h3[:], in0=h2[:, :, 0:64],
                                            in1=h2[:, :, 64:128], op=Alu.max)
                    h4 = swork.tile([P, 16, 32], bf16, tag="h4")
                    nc.vector.tensor_tensor(out=h4[:], in0=h3[:, :, 0:32],
                                            in1=h3[:, :, 32:64], op=Alu.max)
                    tmin = swork.tile([P, 16], bf16, tag="tmin")
                    nc.vector.tensor_reduce(out=tmin[:], in_=h4[:], axis=Ax.X,
                                            op=Alu.max)
                    nc.vector.tensor_reduce(out=rowmax8[:, ib:ib + 1],
                                            in_=tmin[:], axis=Ax.X, op=Alu.max)
                    c16 = swork.tile([P, 16], f32, tag="c16")
                    nc.vector.scalar_tensor_tensor(
                        out=c16[:], in0=tmin[:], scalar=rowmax8[:, ib:ib + 1],
                        in1=iota16MB[:], op0=Alu.is_equal, op1=Alu.mult)
                    tst = swork.tile([P, 1], f32, tag="tst")
                    nc.vector.tensor_reduce(out=tst[:], in_=c16[:], axis=Ax.X,
                                            op=Alu.min)
                    nc.vector.tensor_copy(tstar8[:, ib:ib + 1], tst[:])
                    # spill slab and gather the winning tile; the last two
                    # spills pace the post-loop tail, so split them across
                    # the SP ring and a Pool SWDGE lane to halve latency
                    if ib < IB - 2:
                        eng = (nc.sync, nc.gpsimd)[ib % 2]
                        eng.dma_start(
                            dist_dram.ap()[ib * P:(ib + 1) * P, :], slab[:])
                    else:
                        nc.sync.dma_start(
                            dist_dram.ap()[ib * P:(ib + 1) * P, 0:4096],
                            slab[:, 0:4096])
                        nc.gpsimd.dma_start(
                            dist_dram.ap()[ib * P:(ib + 1) * P, 4096:8192],
                            slab[:, 4096:8192])
                    emit_wg(ib)
                    # software-pipelined tails
                    if ib == 1:
                        cose = emit_edge_b()
                    if ib == 3:
                        emit_edge_c(cose)
                    if ib >= 2:
                        emit_tail(ib - 2)
                    if ib == IB - 1:
                        emit_tail(IB - 2)
                        emit_normal(0, IB - 2)

            # ---- column-max finale: PE transposes + DVE reduces ----
            colmaxT = cpool.tile([P, M // P], f32)
            with tc.tile_pool(name="psumt", bufs=2, space="PSUM") as ppt:
                for g in range(16):
                    t_ps = ppt.tile([P, 512], bf16, tag="tps")
                    for k in range(4):
                        ch = g * 4 + k
                        nc.tensor.transpose(t_ps[:, k * P:(k + 1) * P],
                                            colacc[:, ch * P:(ch + 1) * P],
                                            identb[:])
                    nc.vector.tensor_reduce(
                        out=colmaxT[:, g * 4:(g + 1) * 4],
                        in_=t_ps[:].rearrange("p (k c) -> p k c", k=4),
                        axis=Ax.X, op=Alu.max)
            nc.vector.tensor_scalar(out=colmaxT[:], in0=colmaxT[:],
                                    scalar1=-1.0, scalar2=None, op0=Alu.mult)
            nc.sync.dma_start(colmax_o.ap(), colmaxT[:])

            emit_tail(IB - 1)

            # rowmin output = -rowmax
            rowneg = ssm.tile([P, IB], f32)
            nc.vector.tensor_scalar(out=rowneg[:], in0=rowmax8[:],
                                    scalar1=-1.0, scalar2=None, op0=Alu.mult)
            nc.sync.dma_start(rowmin_o.ap(), rowneg[:])

            # ---- normal consistency: last two columns + combine ----
            emit_normal(IB - 2, IB)
            nc.scalar.activation(cosv[:], cosv[:], Act.Abs)
            sabs = ssm.tile([P, 1], f32)
            nc.vector.tensor_reduce(out=sabs[:], in_=cosv[:], axis=Ax.X,
                                    op=Alu.add)
            nc.sync.dma_start(sabs_o.ap(), sabs[:])

    nc.compile()
    return nc


def _host_edge_terms(verts, faces):
    """Exact numpy port of reference _edge_sharpness + _watertight."""
    v = verts.astype(np.float32)
    f = faces.astype(np.int64)
    v0, v1, v2 = v[f[:, 0]], v[f[:, 1]], v[f[:, 2]]
    n = np.cross(v1 - v0, v2 - v0)
    degen = ((np.abs(n).sum(-1) == 0.0) & (v1 != v0).any(-1) & (v2 != v0).any(-1))
    n[degen] = np.array([1.0, 0.0, 0.0], n.dtype)
    nn = np.maximum(np.linalg.norm(n, axis=-1, keepdims=True), EPS_NRM)
    normals = (n / nn).astype(np.float32)

    a = f
    b = np.roll(f, -1, axis=1)
    lo = np.minimum(a, b).reshape(-1)
    hi = np.maximum(a, b).reshape(-1)
    keys = lo * V + hi
    face_ids = np.repeat(np.arange(f.shape[0], dtype=np.int64), 3)
    order = np.argsort(keys, kind="stable")
    sk = keys[order]
    sf = face_ids[order]
    run_start = np.concatenate([[True], sk[1:] != sk[:-1]])
    eq_next = np.concatenate([sk[:-1] == sk[1:], [False]])
    rs_pad = np.concatenate([run_start, [True, True]])
    pair2 = run_start & eq_next & rs_pad[2:]

    sf_next = np.roll(sf, -1)
    cos = (normals[sf] * normals[sf_next]).sum(-1)
    terms = np.maximum(cos - DIHEDRAL_THRESHOLD, 0.0)
    cnt = pair2.sum()
    edge = float((terms * pair2).sum() / max(cnt, 1)) if cnt > 0 else 0.0

    total = run_start.sum()
    bad = total - pair2.sum()
    wt = float(bad) / float(max(total, 1)) if total > 0 else 0.0
    return np.float32(edge), np.float32(wt)


def _edge_host_inputs(verts, faces):
    """Host provides ORDERING + gathered layout only (lexsort + indexing);
    the device verifies sortedness and does all the arithmetic."""
    a = faces.reshape(-1).astype(np.int32)
    b = np.roll(faces, -1, axis=1).reshape(-1).astype(np.int32)
    lo = np.minimum(a, b)
    hi = np.maximum(a, b)
    perm = np.lexsort((hi, lo)).astype(np.int32)   # stable key order

    loS = np.full(TEP, 20001, np.int32)
    hiS = np.zeros(TEP, np.int32)
    eidS = np.zeros(TEP, np.int32)
    loS[:TE] = lo[perm]
    hiS[:TE] = hi[perm]
    eidS[:TE] = perm
    vfS = np.zeros((TEP, 9), np.float32)
    vfS[:TE] = verts[faces[perm // 3]].reshape(TE, 9)

    def overlap(arr, lo_sent, hi_sent):
        out = np.empty((P, EWo) + arr.shape[1:], arr.dtype)
        for c in range(EWo):
            i = np.arange(P) * EW + c - 1
            valid = (i >= 0) & (i < TEP)
            out[valid, c] = arr[i[valid]]
            out[~valid, c] = lo_sent if (c == 0) else hi_sent
        return out

    return {
        "elo": overlap(loS, -1, -2),
        "ehi": overlap(hiS, -1, -2),
        "eid": overlap(eidS, 0, 0),
        "vfs": overlap(vfS, 0.0, 0.0),
    }


def _lift_p(pts):
    """[K,3] -> [5,K] rows (x, y, z, |p|^2, 1)."""
    k = pts.shape[0]
    out = np.empty((5, k), np.float32)
    out[0:3] = pts.T
    out[3] = (pts * pts).sum(-1)
    out[4] = 1.0
    return out


def _lift_g_neg(pts):
    """[M,3] -> [5,M] rows (2x, 2y, 2z, -1, -|g|^2): p5 . col = -dist."""
    m = pts.shape[0]
    out = np.empty((5, m), np.float32)
    out[0:3] = 2.0 * pts.T
    out[3] = -1.0
    out[4] = -(pts * pts).sum(-1)
    return out


def kernel(pred_sdf, gt_sdf, extracted_vertices, extracted_faces, gt_vertices,
           gt_faces, pred_points, gt_points, pred_normals, gt_normals):
    global _CACHED_NC
    if _CACHED_NC is None:
        _CACHED_NC = _build_program()
    nc = _CACHED_NC

    pp_full = np.asarray(pred_points, np.float32)[0]     # [N,3]
    gp_full = np.asarray(gt_points, np.float32)[0]       # [M,3]
    pn_full = np.asarray(pred_normals, np.float32)[0]
    gn_full = np.asarray(gt_normals, np.float32)[0]
    ps_full = np.asarray(pred_sdf, np.float32).reshape(-1)
    gs_full = np.asarray(gt_sdf, np.float32).reshape(-1)

    g5 = _lift_g_neg(gp_full)
    gn_pad = np.zeros((M, 4), np.float32)
    gn_pad[:, 0:3] = gn_full
    edge_in = _edge_host_inputs(np.asarray(extracted_vertices, np.float32),
                                np.asarray(extracted_faces))
    in_maps = []
    for c in range(NC_CORES):
        rows = pp_full[c * NPC:(c + 1) * NPC]
        # column order (ib, p): column ib*128+p <-> core row p*8+ib
        p5c = _lift_p(rows)                               # [5, NPC] core-row order
        p5c = p5c.reshape(5, P, IB).transpose(0, 2, 1).reshape(5, NPC).copy()
        in_maps.append({
            "p5": p5c,
            "g5": g5,
            "pn": pn_full[c * NPC:(c + 1) * NPC].copy(),
            "gnrm": gn_pad,
            "ps": ps_full[c * NSC:(c + 1) * NSC].reshape(P, NSC // P).copy(),
            "gs": gs_full[c * NSC:(c + 1) * NSC].reshape(P, NSC // P).copy(),
            # per-core column shard of the sorted edge layout
            **{k: np.ascontiguousarray(v[:, c * EWC:c * EWC + EWoC])
               for k, v in edge_in.items()},
        })

    res = run_bass_kernel_spmd(nc, in_maps, core_ids=list(range(NC_CORES)),
                               trace=KERNEL_TRACE)
    if KERNEL_TRACE and res.exec_time_ns is not None:
        print(f"HW exec time: {res.exec_time_ns} ns")
    if TRACE_SINK is not None and res.instructions_and_trace is not None:
        TRACE_SINK["insts"] = res.instructions_and_trace[0]

    # ---- host combine ----
    rowmin_sum = 0.0
    sabs_sum = 0.0
    sdf_sum = 0.0
    colmin = np.full(M, np.inf, np.float64)
    for c in range(NC_CORES):
        r = res.results[c]
        rowmin_sum += r["rowmin"].astype(np.float64).sum()
        sabs_sum += r["sabs"].astype(np.float64).sum()
        sdf_sum += r["sdfsum"].astype(np.float64).sum()
        colmin = np.minimum(colmin, r["colmax"].astype(np.float64).T.reshape(M))

    sdf_l = SDF_W * sdf_sum / NS
    min_p2g = rowmin_sum / N
    min_g2p = colmin.mean()
    chamfer_l = CHAMFER_W * (min_p2g + min_g2p)
    normal_l = NORMAL_W * (N - sabs_sum) / N

    ep = sum(res.results[c]["epart"].astype(np.float64)
             for c in range(NC_CORES))
    viol = ep[:, 3].sum()
    if viol != 0:
        raise RuntimeError(f"device sort-order verification failed: {viol}")
    total = ep[:, 0].sum() - 1.0      # minus the padding run
    cnt2 = ep[:, 1].sum()
    s2 = ep[:, 2].sum()
    edge = s2 / max(cnt2, 1.0) if cnt2 > 0 else 0.0
    bad = total - cnt2
    wt = bad / max(total, 1.0) if total > 0 else 0.0
    edge_l = EDGE_W * float(edge)
    wt_l = WATERTIGHT_W * float(wt)

    total = sdf_l + chamfer_l + normal_l + edge_l + wt_l
    return (np.float32(sdf_l), np.float32(chamfer_l), np.float32(normal_l),
            np.float32(edge_l), np.float32(wt_l), np.float32(total))


# revision 44
# speedup vs baseline: 1.0041x; 1.0041x over previous
"""Trainium2 Bass kernel for nn_ClearMeshLoss.

Sharding: pred-point axis (N=8192) split 8 ways; each core computes
  - its 1024x8192 slab of the (negated) pairwise sq-dist matrix via PE
    matmuls (K=5 lift, host negates the gt lift so PSUM = -dist),
  - slab staged to SBUF in bf16 (ACT cast) -> DVE 2x throughput,
  - row max (= -row min) via bf16 halving folds to per-tile maxima; the
    winning 512-wide tile is found with an is_equal/iota-min search,
    re-fetched from a DRAM spill by one per-partition indirect DMA, and
    searched for the exact argmax column (software-pipelined across
    i-blocks, two iterations deep),
  - column-max partials accumulated in bf16, reduced across partitions
    via PE transposes + DVE reduces, combined across cores on host,
  - normal-consistency cosines via one indirect-DMA gather of matched
    gt normals,
  - its slice of the SDF L1 sum,
  - edge-sharpness / watertight terms on the gpsimd engine (overlapped
    with the DVE chamfer work): host supplies only a lexsort ORDERING of
    the 120k edge keys (plus gathered per-edge face-vertex layout); the
    device verifies sortedness and computes face normals, dihedral
    cosines, run-length counts, and all sums. A sort-order violation
    raises at runtime.
"""
import numpy as np

import concourse.bass as bass
import concourse.mybir as mybir
import concourse.tile as tile
from concourse import bacc
from concourse.bass_utils import run_bass_kernel_spmd

P = 128
N = 8192          # pred points (total)
M = 8192          # gt points
NC_CORES = 8
NPC = N // NC_CORES          # 1024 pred rows per core
IB = NPC // P                # 8 i-blocks per core
NS = 65536
NSC = NS // NC_CORES         # 8192 sdf elems per core
V = 20000
F = 40000

CHAMFER_W, NORMAL_W, EDGE_W, WATERTIGHT_W, SDF_W = 1.0, 0.5, 0.3, 0.2, 1.0
DIHEDRAL_THRESHOLD = 0.5
EPS_COS = 1e-8
EPS_NRM = 1e-12

# edge pipeline: 3F = 120000 edges padded to 2^17, laid out [128, 1024] with a
# 3-column overlap so run/pair/cos windows never cross partitions
TE = 3 * F                 # 120000 real edges
TEP = 131072               # padded
EW = TEP // P              # 1024 own columns per partition
EWo = EW + 3               # own + 3 overlap columns (host-side full layout)
EWC = EW // NC_CORES       # 128 own columns per partition per core
EWoC = EWC + 3             # per-core slice width

KERNEL_TRACE = False
TRACE_SINK = None
_CACHED_NC = None

f32 = mybir.dt.float32
f32r = mybir.dt.float32r
bf16 = mybir.dt.bfloat16
i32 = mybir.dt.int32
Alu = mybir.AluOpType
Ax = mybir.AxisListType
Act = mybir.ActivationFunctionType


def _build_program():
    from concourse.masks import make_identity

    nc = bacc.Bacc("TRN2", target_bir_lowering=False, debug=False,
                   num_devices=NC_CORES)

    # ---- I/O ----
    p5 = nc.dram_tensor("p5", [5, NPC], f32r, kind="ExternalInput")
    g5 = nc.dram_tensor("g5", [5, M], f32r, kind="ExternalInput")  # negated
    pn = nc.dram_tensor("pn", [NPC, 3], f32, kind="ExternalInput")
    gnrm = nc.dram_tensor("gnrm", [M, 4], f32, kind="ExternalInput")
    ps = nc.dram_tensor("ps", [P, NSC // P], f32, kind="ExternalInput")
    gs = nc.dram_tensor("gs", [P, NSC // P], f32, kind="ExternalInput")

    elo = nc.dram_tensor("elo", [P, EWoC], i32, kind="ExternalInput")
    ehi = nc.dram_tensor("ehi", [P, EWoC], i32, kind="ExternalInput")
    eid = nc.dram_tensor("eid", [P, EWoC], i32, kind="ExternalInput")
    vfs = nc.dram_tensor("vfs", [P, EWoC, 9], f32, kind="ExternalInput")

    rowmin_o = nc.dram_tensor("rowmin", [P, IB], f32, kind="ExternalOutput")
    epart_o = nc.dram_tensor("epart", [P, 4], f32, kind="ExternalOutput")
    sabs_o = nc.dram_tensor("sabs", [P, 1], f32, kind="ExternalOutput")
    colmax_o = nc.dram_tensor("colmax", [P, M // P], f32, kind="ExternalOutput")
    sdfsum_o = nc.dram_tensor("sdfsum", [P, 1], f32, kind="ExternalOutput")

    # DRAM scratch: negated dist slab, bf16, row (ib*P + p) = 8192 cols
    dist_dram = nc.dram_tensor("dist_scratch", [IB * P, M], bf16,
                               kind="Internal")

    with tile.TileContext(nc) as tc:
        with (
            tc.tile_pool(name="const", bufs=1) as cpool,
            tc.tile_pool(name="swork", bufs=2) as swork,
            tc.tile_pool(name="ssm", bufs=2) as ssm,
            tc.tile_pool(name="ep", bufs=1) as ep,
        ):
            # ---- chamfer inputs first: matmuls are the critical path ----
            p5_sb = cpool.tile([5, NPC], f32r)
            nc.sync.dma_start(p5_sb[:], p5.ap())
            g5_sb = cpool.tile([5, M], f32r)
            nc.sync.dma_start(g5_sb[:], g5.ap())
            pn_sb = ssm.tile([P, IB, 3], f32)
            nc.sync.dma_start(pn_sb[:], pn.ap().rearrange("(p q) d -> p q d", p=P))
            ps_sb = ssm.tile([P, NSC // P], f32)
            gs_sb = ssm.tile([P, NSC // P], f32)
            nc.sync.dma_start(ps_sb[:], ps.ap())
            nc.sync.dma_start(gs_sb[:], gs.ap())

            elo_t = ep.tile([P, EWoC], i32)
            ehi_t = ep.tile([P, EWoC], i32)
            eid_t = ep.tile([P, EWoC], i32)
            vfs_t = ep.tile([P, EWoC, 9], f32)
            nc.sync.dma_start(elo_t[:], elo.ap())
            nc.sync.dma_start(ehi_t[:], ehi.ap())
            nc.sync.dma_start(eid_t[:], eid.ap())
            nc.sync.dma_start(vfs_t[:], vfs.ap())

            # ---- constants (gpsimd; cheap, early) ----
            it512_i = cpool.tile([P, 512], i32)
            nc.gpsimd.iota(it512_i[:], [[1, 512]], channel_multiplier=0)
            iota512MB = cpool.tile([P, 512], f32)   # iota - 1024
            nc.gpsimd.tensor_copy(iota512MB[:], it512_i[:])
            nc.gpsimd.tensor_scalar(out=iota512MB[:], in0=iota512MB[:],
                                    scalar1=1024.0, scalar2=None,
                                    op0=Alu.subtract)
            it16_i = cpool.tile([P, 16], i32)
            nc.gpsimd.iota(it16_i[:], [[1, 16]], channel_multiplier=0)
            iota16MB = cpool.tile([P, 16], f32)     # iota - 1024
            nc.gpsimd.tensor_copy(iota16MB[:], it16_i[:])
            nc.gpsimd.tensor_scalar(out=iota16MB[:], in0=iota16MB[:],
                                    scalar1=1024.0, scalar2=None,
                                    op0=Alu.subtract)
            rowb_i = cpool.tile([P, 1], i32)        # p*16 win-row base
            nc.gpsimd.iota(rowb_i[:], [[1, 1]], channel_multiplier=16)
            rowb_f = cpool.tile([P, 1], f32)
            nc.gpsimd.tensor_copy(rowb_f[:], rowb_i[:])
            eps4 = cpool.tile([P, 1], f32)
            nc.gpsimd.memset(eps4[:], 1e-24)
            identf = cpool.tile([P, P], f32)
            make_identity(nc, identf[:])
            identb = cpool.tile([P, P], bf16)
            nc.gpsimd.tensor_copy(identb[:], identf[:])

            # ---- edge phase part A: int/compare prefix on DVE (fills the
            # ---- DVE while the first matmuls run), geometry chain on Pool
            W1 = EWoC - 1  # 130
            dlo = ep.tile([P, W1], i32, tag="ti1")
            nc.vector.tensor_tensor(out=dlo[:], in0=elo_t[:, 1:],
                                    in1=elo_t[:, :-1], op=Alu.not_equal)
            dhi = ep.tile([P, W1], i32, tag="ti2")
            nc.vector.tensor_tensor(out=dhi[:], in0=ehi_t[:, 1:],
                                    in1=ehi_t[:, :-1], op=Alu.not_equal)
            rs = ep.tile([P, W1], i32, tag="rs")
            nc.vector.tensor_tensor(out=rs[:], in0=dlo[:], in1=dhi[:],
                                    op=Alu.logical_or)
            notr = ep.tile([P, W1], i32, tag="ti2")
            nc.vector.tensor_scalar(out=notr[:], in0=rs[:], scalar1=-1,
                                    scalar2=1, op0=Alu.mult, op1=Alu.add)
            p2 = ep.tile([P, EWC], i32, tag="p2")
            nc.vector.tensor_tensor(out=p2[:], in0=rs[:, 0:EWC],
                                    in1=notr[:, 1:EWC + 1], op=Alu.logical_and)
            nc.vector.tensor_tensor(out=p2[:], in0=p2[:], in1=rs[:, 2:EWC + 2],
                                    op=Alu.logical_and)
            totali = ep.tile([P, 1], i32, tag="s1")
            with nc.allow_low_precision(reason="exact small-int counts"):
                nc.vector.tensor_reduce(out=totali[:], in_=rs[:, 0:EWC],
                                        axis=Ax.X, op=Alu.add)
            p2f = ep.tile([P, EWC], f32, tag="p2f")
            nc.vector.tensor_copy(p2f[:], p2[:])

            lt1 = ep.tile([P, EWC], i32, tag="ti1")
            nc.vector.tensor_tensor(out=lt1[:], in0=elo_t[:, 1:EWC + 1],
                                    in1=elo_t[:, 0:EWC], op=Alu.is_lt)
            eq1 = ep.tile([P, EWC], i32, tag="ti3")
            nc.vector.tensor_tensor(out=eq1[:], in0=elo_t[:, 1:EWC + 1],
                                    in1=elo_t[:, 0:EWC], op=Alu.is_equal)
            lt2 = ep.tile([P, EWC], i32, tag="ti2")
            nc.vector.tensor_tensor(out=lt2[:], in0=ehi_t[:, 1:EWC + 1],
                                    in1=ehi_t[:, 0:EWC], op=Alu.is_lt)
            nc.vector.tensor_tensor(out=eq1[:], in0=eq1[:], in1=lt2[:],
                                    op=Alu.logical_and)
            nc.vector.tensor_tensor(out=eq1[:], in0=eq1[:], in1=lt1[:],
                                    op=Alu.logical_or)
            violi = ep.tile([P, 1], i32, tag="s2")
            with nc.allow_low_precision(reason="exact small-int counts"):
                nc.vector.tensor_reduce(out=violi[:], in_=eq1[:], axis=Ax.X,
                                        op=Alu.add)

            eidf = ep.tile([P, EWoC], f32, tag="tf1")
            nc.vector.tensor_copy(eidf[:], eid_t[:])
            nc.vector.tensor_scalar(out=eidf[:], in0=eidf[:], scalar1=-1.0,
                                    scalar2=0.33333334, op0=Alu.add,
                                    op1=Alu.mult)
            fidi = ep.tile([P, EWoC], i32, tag="ti4")
            nc.vector.tensor_copy(fidi[:], eidf[:])
            samef = ep.tile([P, EWC], i32, tag="ti1")
            nc.vector.tensor_tensor(out=samef[:], in0=fidi[:, 1:EWC + 1],
                                    in1=fidi[:, 2:EWC + 2], op=Alu.is_equal)
            samef_f = ep.tile([P, EWC], f32, tag="tf2")
            nc.vector.tensor_copy(samef_f[:], samef[:])
            # XLA-FMA artifact emulation: degenerate face with v1==v2 gets a
            # unit normal in the reference, so a self-paired edge scores 0.5
            eqv = ep.tile([P, EWoC, 3], f32, tag="eq3")
            nc.vector.tensor_tensor(out=eqv[:], in0=vfs_t[:, :, 3:6],
                                    in1=vfs_t[:, :, 6:9], op=Alu.is_equal)
            alleq = ep.tile([P, EWoC], f32, tag="tf3")
            nc.vector.tensor_reduce(out=alleq[:], in_=eqv[:], axis=Ax.X,
                                    op=Alu.min)

            # Pool geometry chain (input/DVE-prefix dependent only)
            ovr = ep.tile([P, EWC], f32, tag="tf4")
            nc.gpsimd.tensor_tensor(out=ovr[:], in0=samef_f[:],
                                    in1=alleq[:, 1:EWC + 1], op=Alu.mult)
            e1t = ep.tile([P, EWoC, 3], f32, tag="e1")
            nc.gpsimd.tensor_tensor(out=e1t[:], in0=vfs_t[:, :, 3:6],
                                    in1=vfs_t[:, :, 0:3], op=Alu.subtract)
            e2t = ep.tile([P, EWoC, 3], f32, tag="e2")
            nc.gpsimd.tensor_tensor(out=e2t[:], in0=vfs_t[:, :, 6:9],
                                    in1=vfs_t[:, :, 0:3], op=Alu.subtract)
            n3 = ep.tile([P, EWoC, 3], f32, tag="n3")
            for k in range(3):
                ka, kb = (k + 1) % 3, (k + 2) % 3
                m1 = ep.tile([P, EWoC], f32, tag="tm1")
                m2 = ep.tile([P, EWoC], f32, tag="tm2")
                nc.gpsimd.tensor_tensor(out=m1[:], in0=e1t[:, :, ka],
                                        in1=e2t[:, :, kb], op=Alu.mult)
                nc.gpsimd.tensor_tensor(out=m2[:], in0=e1t[:, :, kb],
                                        in1=e2t[:, :, ka], op=Alu.mult)
                nc.gpsimd.tensor_tensor(out=n3[:, :, k], in0=m1[:], in1=m2[:],
                                        op=Alu.subtract)
            nsq = ep.tile([P, EWoC], f32, tag="tm3")
            nc.gpsimd.tensor_tensor(out=nsq[:], in0=n3[:, :, 0],
                                    in1=n3[:, :, 0], op=Alu.mult)
            for k in (1, 2):
                mk = ep.tile([P, EWoC], f32, tag="tm1")
                nc.gpsimd.tensor_tensor(out=mk[:], in0=n3[:, :, k],
                                        in1=n3[:, :, k], op=Alu.mult)
                nc.gpsimd.tensor_tensor(out=nsq[:], in0=nsq[:], in1=mk[:],
                                        op=Alu.add)
            # |n1|^2 |n2|^2 product of adjacent entries, then unnormalized dot
            nsqp = ep.tile([P, EWC], f32, tag="tf5")
            nc.gpsimd.tensor_tensor(out=nsqp[:], in0=nsq[:, 1:EWC + 1],
                                    in1=nsq[:, 2:EWC + 2], op=Alu.mult)
            dotu = ep.tile([P, EWC], f32, tag="tf6")
            du1 = ep.tile([P, EWC], f32, tag="tm1")
            nc.gpsimd.tensor_tensor(out=dotu[:], in0=n3[:, 1:EWC + 1, 0],
                                    in1=n3[:, 2:EWC + 2, 0], op=Alu.mult)
            for k in (1, 2):
                nc.gpsimd.tensor_tensor(out=du1[:], in0=n3[:, 1:EWC + 1, k],
                                        in1=n3[:, 2:EWC + 2, k], op=Alu.mult)
                nc.gpsimd.tensor_tensor(out=dotu[:], in0=dotu[:], in1=du1[:],
                                        op=Alu.add)

            # ---- sdf L1 partial (DVE, input-dependent only) ----
            sdiff = ssm.tile([P, NSC // P], f32)
            nc.vector.tensor_tensor(out=sdiff[:], in0=ps_sb[:], in1=gs_sb[:],
                                    op=Alu.subtract)
            sdfsum = ssm.tile([P, 1], f32)
            nc.vector.tensor_reduce(out=sdfsum[:], in_=sdiff[:], axis=Ax.X,
                                    op=Alu.add, apply_absolute_value=True)
            nc.sync.dma_start(sdfsum_o.ap(), sdfsum[:])

            # ---- chamfer loop with software-pipelined argmax tail ----
            colacc = cpool.tile([P, M], bf16)
            rowmax8 = cpool.tile([P, IB], f32)
            tstar8 = cpool.tile([P, IB], f32)
            wins = []
            for i in range(IB):
                win_t = cpool.tile([P, 512], bf16, tag=f"win{i}", name=f"win{i}")
                wins.append(win_t)
            matched4 = ssm.tile([P, IB, 4], f32)
            matched = matched4[:, :, 0:3]
            dotn = ssm.tile([P, IB], f32)
            pnn = ssm.tile([P, IB], f32)
            gnn = ssm.tile([P, IB], f32)
            cosv = ssm.tile([P, IB], f32)
            tmp3 = ssm.tile([P, IB, 3], f32)

            def emit_normal(lo, hi):
                s = slice(lo, hi)
                nc.vector.tensor_tensor(out=tmp3[:, s, :], in0=pn_sb[:, s, :],
                                        in1=matched[:, s, :], op=Alu.mult)
                nc.vector.tensor_reduce(out=dotn[:, s], in_=tmp3[:, s, :],
                                        axis=Ax.X, op=Alu.add)
                nc.vector.tensor_tensor(out=tmp3[:, s, :], in0=pn_sb[:, s, :],
                                        in1=pn_sb[:, s, :], op=Alu.mult)
                nc.vector.tensor_reduce(out=pnn[:, s], in_=tmp3[:, s, :],
                                        axis=Ax.X, op=Alu.add)
                nc.scalar.activation(pnn[:, s], pnn[:, s], Act.Sqrt)
                nc.vector.tensor_scalar(out=pnn[:, s], in0=pnn[:, s],
                                        scalar1=EPS_COS, scalar2=None,
                                        op0=Alu.max)
                nc.vector.tensor_tensor(out=tmp3[:, s, :], in0=matched[:, s, :],
                                        in1=matched[:, s, :], op=Alu.mult)
                nc.vector.tensor_reduce(out=gnn[:, s], in_=tmp3[:, s, :],
                                        axis=Ax.X, op=Alu.add)
                nc.scalar.activation(gnn[:, s], gnn[:, s], Act.Sqrt)
                nc.vector.tensor_scalar(out=gnn[:, s], in0=gnn[:, s],
                                        scalar1=EPS_COS, scalar2=None,
                                        op0=Alu.max)
                nc.vector.tensor_tensor(out=gnn[:, s], in0=pnn[:, s],
                                        in1=gnn[:, s], op=Alu.mult)
                nc.vector.reciprocal(gnn[:, s], gnn[:, s])
                nc.vector.tensor_tensor(out=cosv[:, s], in0=dotn[:, s],
                                        in1=gnn[:, s], op=Alu.mult)

            def emit_wg(jb):
                # win-tile row = p*16 + t* + 1024 + jb*2048; gather the tile
                ridx_f = swork.tile([P, 1], f32, tag="ridx_f", name="ridx_f")
                nc.vector.scalar_tensor_tensor(
                    out=ridx_f[:], in0=tstar8[:, jb:jb + 1],
                    scalar=float(1024 + jb * 2048), in1=rowb_f[:],
                    op0=Alu.add, op1=Alu.add)
                ridx_i = swork.tile([P, 1], i32, tag="ridx_i", name="ridx_i")
                nc.vector.tensor_copy(ridx_i[:], ridx_f[:])
                nc.gpsimd.indirect_dma_start(
                    out=wins[jb][:], out_offset=None,
                    in_=dist_dram.ap().rearrange("r (t c) -> (r t) c", c=512),
                    in_offset=bass.IndirectOffsetOnAxis(ap=ridx_i[:, :1],
                                                        axis=0))

            def emit_tail(jb):
                # w* within winning tile; nnidx; gather matched gt normal
                cw = swork.tile([P, 512], f32, tag="cw", name="cw")
                nc.vector.scalar_tensor_tensor(
                    out=cw[:], in0=wins[jb][:],
                    scalar=rowmax8[:, jb:jb + 1], in1=iota512MB[:],
                    op0=Alu.is_equal, op1=Alu.mult)
                ws = swork.tile([P, 1], f32, tag="ws", name="ws")
                nc.vector.tensor_reduce(out=ws[:], in_=cw[:], axis=Ax.X,
                                        op=Alu.min)
                # nnidx = t*_raw*512 + w*_raw + (1024*512 + 1024)
                nnf = swork.tile([P, 1], f32, tag="nnf", name="nnf")
                nc.vector.scalar_tensor_tensor(
                    out=nnf[:], in0=tstar8[:, jb:jb + 1], scalar=512.0,
                    in1=ws[:], op0=Alu.mult, op1=Alu.add)
                nc.vector.tensor_scalar(out=nnf[:], in0=nnf[:],
                                        scalar1=float(1024 * 512 + 1024),
                                        scalar2=None, op0=Alu.add)
                nni = swork.tile([P, 1], i32, tag="nni", name="nni")
                nc.vector.tensor_copy(nni[:], nnf[:])
                mg = swork.tile([P, 4], f32, tag=f"mg{jb}", name="mg")
                nc.gpsimd.indirect_dma_start(
                    out=mg[:], out_offset=None, in_=gnrm.ap(),
                    in_offset=bass.IndirectOffsetOnAxis(ap=nni[:, :1], axis=0))
                nc.vector.tensor_copy(matched4[:, jb:jb + 1, :], mg[:, None, :])

            def emit_edge_b():
                # sqrt(|n1|^2 |n2|^2 + eps^4): eps bias keeps 1/x finite for
                # exactly-degenerate faces (reference clamps |n| to 1e-12)
                snp = ep.tile([P, EWC], f32, tag="tf7", name="snp")
                nc.scalar.activation(snp[:], nsqp[:], Act.Sqrt,
                                     bias=eps4[:, :1])
                rec = ep.tile([P, EWC], f32, tag="tf8", name="rec")
                nc.vector.reciprocal(rec[:], snp[:])
                cose = ep.tile([P, EWC], f32, tag="tf7")
                nc.gpsimd.tensor_tensor(out=cose[:], in0=dotu[:], in1=rec[:],
                                        op=Alu.mult)
                return cose

            def emit_edge_c(cose):
                cosa = ep.tile([P, EWC], f32, tag="tf1")
                nc.vector.tensor_scalar(out=cosa[:], in0=cose[:], scalar1=-0.5,
                                        scalar2=0.0, op0=Alu.add, op1=Alu.max)
                d5 = ep.tile([P, EWC], f32, tag="tf3")
                nc.vector.tensor_scalar(out=d5[:], in0=cosa[:], scalar1=-1.0,
                                        scalar2=0.5, op0=Alu.mult, op1=Alu.add)
                nc.vector.tensor_tensor(out=d5[:], in0=d5[:], in1=ovr[:],
                                        op=Alu.mult)
                nc.vector.tensor_tensor(out=cosa[:], in0=cosa[:], in1=d5[:],
                                        op=Alu.add)
                nc.vector.tensor_tensor(out=cosa[:], in0=cosa[:], in1=p2f[:],
                                        op=Alu.mult)
                spart = ep.tile([P, 1], f32, tag="s3")
                nc.vector.tensor_reduce(out=spart[:], in_=cosa[:], axis=Ax.X,
                                        op=Alu.add)
                cnt2p = ep.tile([P, 1], f32, tag="s4")
                nc.vector.tensor_reduce(out=cnt2p[:], in_=p2f[:], axis=Ax.X,
                                        op=Alu.add)
                epk = ep.tile([P, 4], f32, tag="s5")
                nc.vector.tensor_copy(epk[:, 0:1], totali[:])
                nc.vector.tensor_copy(epk[:, 1:2], cnt2p[:])
                nc.vector.tensor_copy(epk[:, 2:3], spart[:])
                nc.vector.tensor_copy(epk[:, 3:4], violi[:])
                nc.sync.dma_start(epart_o.ap(), epk[:])

            cose = None
            with (
                tc.tile_pool(name="sbig", bufs=4) as sbig,
                tc.tile_pool(name="psum", bufs=2, space="PSUM") as pp,
            ):
                for ib in range(IB):
                    slab = sbig.tile([P, M], bf16, tag="slab")
                    h1 = swork.tile([P, 16, 256], bf16, tag="h1", name="h1")
                    for q in range(4):
                        d_ps = pp.tile([P, 2048], f32, tag="dps")
                        for s in range(4):
                            j0 = (q * 4 + s) * 512
                            nc.tensor.matmul(d_ps[:, s * 512:(s + 1) * 512],
                                             lhsT=p5_sb[:, ib * P:(ib + 1) * P],
                                             rhs=g5_sb[:, j0:j0 + 512],
                                             start=True, stop=True)
                        nc.scalar.activation(slab[:, q * 2048:(q + 1) * 2048],
                                             d_ps[:], Act.Copy)
                        if ib == 0:
                            nc.vector.tensor_copy(
                                colacc[:, q * 2048:(q + 1) * 2048],
                                slab[:, q * 2048:(q + 1) * 2048])
                        else:
                            nc.vector.tensor_tensor(
                                out=colacc[:, q * 2048:(q + 1) * 2048],
                                in0=colacc[:, q * 2048:(q + 1) * 2048],
                                in1=slab[:, q * 2048:(q + 1) * 2048],
                                op=Alu.max)
                    # per-tile maxima via halving folds (bf16 2x)
                    s3 = slab[:].rearrange("p (t c) -> p t c", t=16)
                    nc.vector.tensor_tensor(out=h1[:], in0=s3[:, :, 0:256],
                                            in1=s3[:, :, 256:512], op=Alu.max)
                    h2 = swork.tile([P, 16, 128], bf16, tag="h2")
                    nc.vector.tensor_tensor(out=h2[:], in0=h1[:, :, 0:128],
                                            in1=h1[:, :, 128:256], op=Alu.max)
                    h3 = swork.tile([P, 16, 64], bf16, tag="h3")
                    nc.vector.tensor_tensor(out=h3[:], in0=h2[:, :, 0:64],
                                            in1=h2[:, :, 64:128], op=Alu.max)
                    h4 = swork.tile([P, 16, 32], bf16, tag="h4")
                    nc.vector.tensor_tensor(out=h4[:], in0=h3[:, :, 0:32],
                                            in1=h3[:, :, 32:64], op=Alu.max)
                    tmin = swork.tile([P, 16], bf16, tag="tmin")
                    nc.vector.tensor_reduce(out=tmin[:], in_=h4[:], axis=Ax.X,
                                            op=Alu.max)
                    nc.vector.tensor_reduce(out=rowmax8[:, ib:ib + 1],
                                            in_=tmin[:], axis=Ax.X, op=Alu.max)
                    c16 = swork.tile([P, 16], f32, tag="c16")
                    nc.vector.scalar_tensor_tensor(
                        out=c16[:], in0=tmin[:], scalar=rowmax8[:, ib:ib + 1],
                        in1=iota16MB[:], op0=Alu.is_equal, op1=Alu.mult)
                    tst = swork.tile([P, 1], f32, tag="tst")
                    nc.vector.tensor_reduce(out=tst[:], in_=c16[:], axis=Ax.X,
                                            op=Alu.min)
                    nc.vector.tensor_copy(tstar8[:, ib:ib + 1], tst[:])
                    # spill slab and gather the winning tile; the last two
                    # spills pace the post-loop tail, so split them across
                    # the SP ring and a Pool SWDGE lane to halve latency
                    if ib < IB - 2:
                        nc.sync.dma_start(
                            dist_dram.ap()[ib * P:(ib + 1) * P, :], slab[:])
                    else:
                        nc.sync.dma_start(
                            dist_dram.ap()[ib * P:(ib + 1) * P, 0:4096],
                            slab[:, 0:4096])
                        nc.gpsimd.dma_start(
                            dist_dram.ap()[ib * P:(ib + 1) * P, 4096:8192],
                            slab[:, 4096:8192])
                    emit_wg(ib)
                    # software-pipelined tails
                    if ib == 1:
                        cose = emit_edge_b()
                    if ib == 3:
                        emit_edge_c(cose)
                    if ib >= 2:
                        emit_tail(ib - 2)
                    if ib == IB - 1:
                        emit_tail(IB - 2)
                        emit_normal(0, IB - 2)

            # ---- column-max finale: PE transposes + DVE reduces ----
            colmaxT = cpool.tile([P, M // P], f32)
            with tc.tile_pool(name="psumt", bufs=2, space="PSUM") as ppt:
                for g in range(16):
                    t_ps = ppt.tile([P, 512], bf16, tag="tps")
                    for k in range(4):
                        ch = g * 4 + k
                        nc.tensor.transpose(t_ps[:, k * P:(k + 1) * P],
                                            colacc[:, ch * P:(ch + 1) * P],
                                            identb[:])
                    nc.vector.tensor_reduce(
                        out=colmaxT[:, g * 4:(g + 1) * 4],
                        in_=t_ps[:].rearrange("p (k c) -> p k c", k=4),
                        axis=Ax.X, op=Alu.max)
            nc.vector.tensor_scalar(out=colmaxT[:], in0=colmaxT[:],
                                    scalar1=-1.0, scalar2=None, op0=Alu.mult)
            nc.sync.dma_start(colmax_o.ap(), colmaxT[:])

            emit_tail(IB - 1)

            # rowmin output = -rowmax
            rowneg = ssm.tile([P, IB], f32)
            nc.vector.tensor_scalar(out=rowneg[:], in0=rowmax8[:],
                                    scalar1=-1.0, scalar2=None, op0=Alu.mult)
            nc.sync.dma_start(rowmin_o.ap(), rowneg[:])

            # ---- normal consistency: last two columns + combine ----
            emit_normal(IB - 2, IB)
            nc.scalar.activation(cosv[:], cosv[:], Act.Abs)
            sabs = ssm.tile([P, 1], f32)
            nc.vector.tensor_reduce(out=sabs[:], in_=cosv[:], axis=Ax.X,
                                    op=Alu.add)
            nc.sync.dma_start(sabs_o.ap(), sabs[:])

    nc.compile()
    return nc


def _host_edge_terms(verts, faces):
    """Exact numpy port of reference _edge_sharpness + _watertight."""
    v = verts.astype(np.float32)
    f = faces.astype(np.int64)
    v0, v1, v2 = v[f[:, 0]], v[f[:, 1]], v[f[:, 2]]
    n = np.cross(v1 - v0, v2 - v0)
    degen = ((np.abs(n).sum(-1) == 0.0) & (v1 != v0).any(-1) & (v2 != v0).any(-1))
    n[degen] = np.array([1.0, 0.0, 0.0], n.dtype)
    nn = np.maximum(np.linalg.norm(n, axis=-1, keepdims=True), EPS_NRM)
    normals = (n / nn).astype(np.float32)

    a = f
    b = np.roll(f, -1, axis=1)
    lo = np.minimum(a, b).reshape(-1)
    hi = np.maximum(a, b).reshape(-1)
    keys = lo * V + hi
    face_ids = np.repeat(np.arange(f.shape[0], dtype=np.int64), 3)
    order = np.argsort(keys, kind="stable")
    sk = keys[order]
    sf = face_ids[order]
    run_start = np.concatenate([[True], sk[1:] != sk[:-1]])
    eq_next = np.concatenate([sk[:-1] == sk[1:], [False]])
    rs_pad = np.concatenate([run_start, [True, True]])
    pair2 = run_start & eq_next & rs_pad[2:]

    sf_next = np.roll(sf, -1)
    cos = (normals[sf] * normals[sf_next]).sum(-1)
    terms = np.maximum(cos - DIHEDRAL_THRESHOLD, 0.0)
    cnt = pair2.sum()
    edge = float((terms * pair2).sum() / max(cnt, 1)) if cnt > 0 else 0.0

    total = run_start.sum()
    bad = total - pair2.sum()
    wt = float(bad) / float(max(total, 1)) if total > 0 else 0.0
    return np.float32(edge), np.float32(wt)


def _edge_host_inputs(verts, faces):
    """Host provides ORDERING + gathered layout only (lexsort + indexing);
    the device verifies sortedness and does all the arithmetic."""
    a = faces.reshape(-1).astype(np.int32)
    b = np.roll(faces, -1, axis=1).reshape(-1).astype(np.int32)
    lo = np.minimum(a, b)
    hi = np.maximum(a, b)
    perm = np.lexsort((hi, lo)).astype(np.int32)   # stable key order

    loS = np.full(TEP, 20001, np.int32)
    hiS = np.zeros(TEP, np.int32)
    eidS = np.zeros(TEP, np.int32)
    loS[:TE] = lo[perm]
    hiS[:TE] = hi[perm]
    eidS[:TE] = perm
    vfS = np.zeros((TEP, 9), np.float32)
    vfS[:TE] = verts[faces[perm // 3]].reshape(TE, 9)

    def overlap(arr, lo_sent, hi_sent):
        out = np.empty((P, EWo) + arr.shape[1:], arr.dtype)
        for c in range(EWo):
            i = np.arange(P) * EW + c - 1
            valid = (i >= 0) & (i < TEP)
            out[valid, c] = arr[i[valid]]
            out[~valid, c] = lo_sent if (c == 0) else hi_sent
        return out

    return {
        "elo": overlap(loS, -1, -2),
        "ehi": overlap(hiS, -1, -2),
        "eid": overlap(eidS, 0, 0),
        "vfs": overlap(vfS, 0.0, 0.0),
    }


def _lift_p(pts):
    """[K,3] -> [5,K] rows (x, y, z, |p|^2, 1)."""
    k = pts.shape[0]
    out = np.empty((5, k), np.float32)
    out[0:3] = pts.T
    out[3] = (pts * pts).sum(-1)
    out[4] = 1.0
    return out


def _lift_g_neg(pts):
    """[M,3] -> [5,M] rows (2x, 2y, 2z, -1, -|g|^2): p5 . col = -dist."""
    m = pts.shape[0]
    out = np.empty((5, m), np.float32)
    out[0:3] = 2.0 * pts.T
    out[3] = -1.0
    out[4] = -(pts * pts).sum(-1)
    return out


def kernel(pred_sdf, gt_sdf, extracted_vertices, extracted_faces, gt_vertices,
           gt_faces, pred_points, gt_points, pred_normals, gt_normals):
    global _CACHED_NC
    if _CACHED_NC is None:
        _CACHED_NC = _build_program()
    nc = _CACHED_NC

    pp_full = np.asarray(pred_points, np.float32)[0]     # [N,3]
    gp_full = np.asarray(gt_points, np.float32)[0]       # [M,3]
    pn_full = np.asarray(pred_normals, np.float32)[0]
    gn_full = np.asarray(gt_normals, np.float32)[0]
    ps_full = np.asarray(pred_sdf, np.float32).reshape(-1)
    gs_full = np.asarray(gt_sdf, np.float32).reshape(-1)

    g5 = _lift_g_neg(gp_full)
    gn_pad = np.zeros((M, 4), np.float32)
    gn_pad[:, 0:3] = gn_full
    edge_in = _edge_host_inputs(np.asarray(extracted_vertices, np.float32),
                                np.asarray(extracted_faces))
    in_maps = []
    for c in range(NC_CORES):
        rows = pp_full[c * NPC:(c + 1) * NPC]
        # column order (ib, p): column ib*128+p <-> core row p*8+ib
        p5c = _lift_p(rows)                               # [5, NPC] core-row order
        p5c = p5c.reshape(5, P, IB).transpose(0, 2, 1).reshape(5, NPC).copy()
        in_maps.append({
            "p5": p5c,
            "g5": g5,
            "pn": pn_full[c * NPC:(c + 1) * NPC].copy(),
            "gnrm": gn_pad,
            "ps": ps_full[c * NSC:(c + 1) * NSC].reshape(P, NSC // P).copy(),
            "gs": gs_full[c * NSC:(c + 1) * NSC].reshape(P, NSC // P).copy(),
            # per-core column shard of the sorted edge layout
            **{k: np.ascontiguousarray(v[:, c * EWC:c * EWC + EWoC])
               for k, v in edge_in.items()},
        })

    res = run_bass_kernel_spmd(nc, in_maps, core_ids=list(range(NC_CORES)),
                               trace=KERNEL_TRACE)
    if KERNEL_TRACE and res.exec_time_ns is not None:
        print(f"HW exec time: {res.exec_time_ns} ns")
    if TRACE_SINK is not None and res.instructions_and_trace is not None:
        TRACE_SINK["insts"] = res.instructions_and_trace[0]

    # ---- host combine ----
    rowmin_sum = 0.0
    sabs_sum = 0.0
    sdf_sum = 0.0
    colmin = np.full(M, np.inf, np.float64)
    for c in range(NC_CORES):
        r = res.results[c]
        rowmin_sum += r["rowmin"].astype(np.float64).sum()
        sabs_sum += r["sabs"].astype(np.float64).sum()
        sdf_sum += r["sdfsum"].astype(np.float64).sum()
        colmin = np.minimum(colmin, r["colmax"].astype(np.float64).T.reshape(M))

    sdf_l = SDF_W * sdf_sum / NS
    min_p2g = rowmin_sum / N
    min_g2p = colmin.mean()
    chamfer_l = CHAMFER_W * (min_p2g + min_g2p)
    normal_l = NORMAL_W * (N - sabs_sum) / N

    ep = sum(res.results[c]["epart"].astype(np.float64)
             for c in range(NC_CORES))
    viol = ep[:, 3].sum()
    if viol != 0:
        raise RuntimeError(f"device sort-order verification failed: {viol}")
    total = ep[:, 0].sum() - 1.0      # minus the padding run
    cnt2 = ep[:, 1].sum()
    s2 = ep[:, 2].sum()
    edge = s2 / max(cnt2, 1.0) if cnt2 > 0 else 0.0
    bad = total - cnt2
    wt = bad / max(total, 1.0) if total > 0 else 0.0
    edge_l = EDGE_W * float(edge)
    wt_l = WATERTIGHT_W * float(wt)

    total = sdf_l + chamfer_l + normal_l + edge_l + wt_l
    return (np.float32(sdf_l), np.float32(chamfer_l), np.float32(normal_l),
            np.float32(edge_l), np.float32(wt_l), np.float32(total))
